# revision 4
# baseline (speedup 1.0000x reference)
"""Depthwise cross-correlation (DepthwiseRPN) on 8 TRN2 NeuronCores.

Reference op:
  z_f: [B=128, C=256, 7, 7]   per-(b,c) kernels
  x_f: [B=128, C=256, 31, 31] search windows
  out: [B=128, C=256, 25, 25] valid cross-correlation per (b,c)

Sharding: pure data-parallel over B (16 batches per core); per core the
4096 (b,c) channels form 32 groups of 128 partitions, split across three
engine pipelines (measured per-group costs in parens):

  - PE groups (22, ~13.7us): per-tap diagonal matmul, 49 taps accumulate
    in PSUM (500+125 col split across 2 banks).  Runs at the measured
    floor of ~279ns/tap (625 ingest cycles + one weight-change penalty);
    psum pools use all 8 banks (4+4) so matmuls run three groups ahead
    of their evacuations and never stall on ScalarE.
  - DVE groups (4, ~41us): serial fused MAC via the AFFINE_THEN_ADD
    custom DVE op (1x-only): acc = x_win*z_tap + acc, fp32 accumulator;
    stride-32 x rows with an odd-shifted copy keep reads 4B-aligned.
  - AG groups (6, ACT ~45us + DVE ~19us): ScalarE mult (activation Copy
    with per-partition scale) into 626-strided bf16 planes; DVE folds
    each 8-plane chunk with three halving flat adds in 2x mode, chunk
    sums collected into a second 8-plane tile folded the same way.

Emission interleaves pipelines with fractional pacing (front-loaded 15%)
so every engine finishes with the matmuls; evacuations are emitted ahead
of AG products in the ScalarE queue.  Ten scratch warmup matmuls during
the first DMA wait hold the PE HAM clock at 2.4 GHz.  PE/AG outputs
store as bf16 (host converts back); DVE groups keep fp32 accumulators.
Diag lhsT matrices are precomputed host-side (bf16).

Measured: ~319us HW exec (baseline 334us), max rel err ~7e-3 vs fp32.

Engine-balance notes (measured): DVE custom ops and ACTIVATE have no 2x
mode; tensor_tensor flat bf16 adds do (0.58ns/elem).  GpSimd elementwise
shares DVE's SBUF port (concurrent use serializes both ~6x) — unusable.
fp8 DoubleRow matmul fails the 2e-2 precision gate.  The 22/4/6 split is
the LP optimum for these rates; the span is PE-bound at its issue floor.
"""

import numpy as np
import ml_dtypes

import concourse.bass as bass
import concourse.mybir as mybir
import concourse.tile as tile
from concourse import bacc
from concourse.bass_utils import run_bass_kernel_spmd

B, C = 128, 256
HX, WX = 31, 31
HZ, WZ = 7, 7
HO, WO = HX - HZ + 1, WX - WZ + 1  # 25, 25
NCORES = 8
BPC = B // NCORES         # batches per core = 16
Q = BPC * C               # (b,c) channels per core = 4096
G = Q // 128              # groups of 128 channels = 32
NX = HX * WX              # 961
NO = HO * WO              # 625
NT = HZ * WZ              # 49 taps
ROWS_A = 20               # psum chunk A rows (20*25=500 <= 512)
ROWS_B = HO - ROWS_A      # 5 rows (125)
PL = 626                  # bf16 plane stride (even -> 2x adds)

# channel-group split across engines
G_PE = 22                 # TensorE diag-matmul groups
G_DVE = 4                 # DVE fused-MAC (AFFINE_THEN_ADD) groups
G_AG = G - G_PE - G_DVE   # 6: ACT-mult + DVE tree-add groups

BF16 = ml_dtypes.bfloat16

_built = {}


def _ensure_ntff_hook():
    """Install the axon NTFF profiling hook if the container's antenv stub
    lacks it (needed only for trace=True local profiling runs)."""
    import contextlib
    import ctypes
    import sys
    import types

    try:
        from antenv.axon_hooks import get_axon_ntff_profile_hook  # noqa: F401

        return True
    except ImportError:
        pass
    so_path = "/opt/axon/libaxon_pjrt.so"
    try:
        lib = ctypes.CDLL(so_path)
    except OSError:
        return False
    if not hasattr(lib, "axon_start_nrt_profile"):
        return False
    lib.axon_start_nrt_profile.argtypes = [
        ctypes.POINTER(ctypes.c_int64),
        ctypes.c_size_t,
    ]
    lib.axon_start_nrt_profile.restype = ctypes.c_int64
    lib.axon_stop_nrt_profile.argtypes = [ctypes.c_char_p]
    lib.axon_stop_nrt_profile.restype = ctypes.c_int64

    @contextlib.contextmanager
    def _hook(output_dir, device_ids):
        import jax

        jax.devices()
        if device_ids:
            ids = (ctypes.c_int64 * len(device_ids))(*device_ids)
            rc = lib.axon_start_nrt_profile(ids, len(device_ids))
        else:
            rc = lib.axon_start_nrt_profile(None, 0)
        if rc != 0:
            raise RuntimeError(f"axon_start_nrt_profile rc={rc}")
        try:
            yield
        finally:
            n = lib.axon_stop_nrt_profile(str(output_dir).encode())
            print(f"profile: {n} file(s) written to {output_dir}", file=sys.stderr)

    state = {"hook": _hook}
    mod = types.ModuleType("antenv.axon_hooks")
    mod.get_axon_ntff_profile_hook = lambda: state["hook"]
    mod.set_axon_ntff_profile_hook = lambda h: state.update(hook=h)
    import antenv

    sys.modules["antenv.axon_hooks"] = mod
    antenv.axon_hooks = mod
    return True


def _emit_pe_matmuls(nc, pools, x_d, zd_d, g):
    """One PE group's DMAs + 49 tap matmuls; psum tiles returned for a
    deferred evacuation."""
    xp, zp, psA, psB = pools["xp"], pools["zp"], pools["psA"], pools["psB"]
    x_sb = xp.tile([128, HX, WX], mybir.dt.bfloat16, name=f"xpe{g}", tag="xpe", bufs=4)
    zd_sb = zp.tile([128, NT, 128], mybir.dt.bfloat16, name=f"zd{g}", tag="zd", bufs=3)
    xg = x_d[g].rearrange("p (h w) -> p h w", h=HX)
    if g == 0:
        # split the first loads so tap-0's operands (weights chunk +
        # x rows 0..24) land before the rest of the tile streams in
        nc.sync.dma_start(out=x_sb[:, 0:25], in_=xg[:, 0:25])
        nc.sync.dma_start(out=zd_sb[:, 0:2], in_=zd_d[g][:, 0:2])
        nc.sync.dma_start(out=zd_sb[:, 2:12], in_=zd_d[g][:, 2:12])
        nc.sync.dma_start(out=x_sb[:, 25:], in_=xg[:, 25:])
        nc.sync.dma_start(out=zd_sb[:, 12:], in_=zd_d[g][:, 12:])
    else:
        nc.sync.dma_start(out=x_sb, in_=xg)
        nc.sync.dma_start(out=zd_sb, in_=zd_d[g])

    pA = psA.tile([128, ROWS_A * WO], mybir.dt.float32, name=f"pA{g}", tag="pA", bufs=4)
    pB = psB.tile([128, ROWS_B * WO], mybir.dt.float32, name=f"pB{g}", tag="pB", bufs=4)
    for t in range(NT):
        u, v = divmod(t, WZ)
        lhsT = zd_sb[:, t, :]
        nc.tensor.matmul(
            pA[:, :], lhsT, x_sb[:, u : u + ROWS_A, v : v + WO],
            start=(t == 0), stop=(t == NT - 1),
        )
        nc.tensor.matmul(
            pB[:, :], lhsT, x_sb[:, ROWS_A + u : ROWS_A + u + ROWS_B, v : v + WO],
            start=(t == 0), stop=(t == NT - 1),
        )
    return pA, pB


def _emit_pe_evac(nc, pools, out_d, g, pA, pB):
    op = pools["op"]
    out_sb = op.tile([128, NO], mybir.dt.bfloat16, name=f"ope{g}", tag="ope", bufs=4)
    # ScalarE is closest to PSUM; keep DVE free for its MAC pipeline.
    # Store each half as soon as its copy lands so the final group's
    # store overlaps the second copy.
    nc.scalar.copy(out=out_sb[:, : ROWS_A * WO], in_=pA[:, :])
    nc.sync.dma_start(out=out_d[g][:, : ROWS_A * WO], in_=out_sb[:, : ROWS_A * WO])
    nc.scalar.copy(out=out_sb[:, ROWS_A * WO :], in_=pB[:, :])
    nc.sync.dma_start(out=out_d[g][:, ROWS_A * WO :], in_=out_sb[:, ROWS_A * WO :])


def _gen_dve_groups(nc, pools, x_flat, zf_sb, outf_d, groups,
                    taps_per_yield=4):
    """Generator: DVE fused-MAC pipeline over `groups`, yielding every few
    taps so the driver can interleave AG tree adds into DVE's stream."""
    xp, op = pools["xv"], pools["ov"]
    for g in groups:
        # stride-32 rows + an odd-shifted copy keep every window read
        # 4B-aligned (bf16 reads at odd element offsets run ~2x slower)
        x_e = xp.tile([128, HX, 32], mybir.dt.bfloat16, name=f"xdve{g}", tag="xdve", bufs=2)
        x_o = xp.tile([128, HX, 32], mybir.dt.bfloat16, name=f"xdvo{g}", tag="xdvo", bufs=2)
        base = g * 128 * NX
        nc.sync.dma_start(
            out=x_e[:, :, 0:WX],
            in_=bass.AP(tensor=x_flat.tensor, offset=base, ap=[[NX, 128], [WX, HX], [1, WX]]),
        )
        nc.sync.dma_start(
            out=x_o[:, :, 0:WX],
            in_=bass.AP(tensor=x_flat.tensor, offset=base + 1, ap=[[NX, 128], [WX, HX], [1, WX]]),
        )

        gz = g - G_PE
        acc = op.tile([128, HO, WO], mybir.dt.float32, name=f"accv{g}", tag="accv", bufs=2)
        for t in range(NT):
            u, v = divmod(t, WZ)
            if v % 2 == 0:
                win = x_e[:, u : u + HO, v : v + WO]
            else:
                win = x_o[:, u : u + HO, v - 1 : v - 1 + WO]
            if t == 0:
                # seed on DVE: acc = win * z  (keeps ScalarE out of this chain)
                nc.vector.tensor_scalar(
                    acc, win, zf_sb[:, gz, 0:1], None, mybir.AluOpType.mult
                )
            else:
                nc.vector.affine_then_add(acc, win, acc, zf_sb[:, gz, t : t + 1], 0.0)
            if (t + 1) % taps_per_yield == 0:
                yield
        nc.sync.dma_start(out=outf_d[gz], in_=acc.rearrange("p h w -> p (h w)"))
        yield


def _gen_ag_groups(nc, pools, x_d, zf_sb, out_d, groups):
    """Generator: ACT computes per-tap products into 626-strided bf16
    planes; DVE folds each 8-plane chunk (and the chunk-sum tile) with
    three halving flat adds in 2x mode.  Yields after each chunk's
    products and once more after its adds so the driver can interleave
    DVE-affine work.  Chunk layout: taps 0..47 in 6 chunks of 8; tap 48
    lands in the sums tile (slot 6); slot 7 is zeroed once."""
    xp, tp, sp, op = pools["xa"], pools["ta"], pools["sa"], pools["oa"]
    for g in groups:
        x_sb = xp.tile([128, HX, WX], mybir.dt.bfloat16, name=f"xag{g}", tag="xag", bufs=2)
        nc.sync.dma_start(out=x_sb, in_=x_d[g].rearrange("p (h w) -> p h w", h=HX))

        gz = g - G_PE
        sums = sp.tile([128, 8, PL], mybir.dt.bfloat16, name=f"sum{g}", tag="sum", bufs=2)
        sflat = sums.rearrange("p a b -> p (a b)")
        nc.vector.memset(sums[:, 7], 0.0)
        # tap 48 product -> sums slot 6
        nc.scalar.activation(
            sums[:, 6, 0:NO].rearrange("p (h w) -> p h w", h=HO),
            x_sb[:, 6 : 6 + HO, 6 : 6 + WO],
            mybir.ActivationFunctionType.Copy,
            bias=0.0, scale=zf_sb[:, gz, 48:49],
        )
        for ci in range(6):
            ptile = tp.tile([128, 8, PL], mybir.dt.bfloat16, name=f"pt{g}_{ci}", tag="pt", bufs=3)
            for k in range(8):
                t = ci * 8 + k
                u, v = divmod(t, WZ)
                nc.scalar.activation(
                    ptile[:, k, 0:NO].rearrange("p (h w) -> p h w", h=HO),
                    x_sb[:, u : u + HO, v : v + WO],
                    mybir.ActivationFunctionType.Copy,
                    bias=0.0, scale=zf_sb[:, gz, t : t + 1],
                )
            yield
            pflat = ptile.rearrange("p a b -> p (a b)")
            nc.vector.tensor_add(pflat[:, 0 : 4 * PL], pflat[:, 0 : 4 * PL], pflat[:, 4 * PL : 8 * PL])
            nc.vector.tensor_add(pflat[:, 0 : 2 * PL], pflat[:, 0 : 2 * PL], pflat[:, 2 * PL : 4 * PL])
            nc.vector.tensor_add(sums[:, ci], pflat[:, 0:PL], pflat[:, PL : 2 * PL])
            yield
        # fold the 8 collected planes (6 chunk sums + tap48 + zero)
        out_sb = op.tile([128, PL], mybir.dt.bfloat16, name=f"oag{g}", tag="oag", bufs=2)
        nc.vector.tensor_add(sflat[:, 0 : 4 * PL], sflat[:, 0 : 4 * PL], sflat[:, 4 * PL : 8 * PL])
        nc.vector.tensor_add(sflat[:, 0 : 2 * PL], sflat[:, 0 : 2 * PL], sflat[:, 2 * PL : 4 * PL])
        nc.vector.tensor_add(out_sb, sflat[:, 0:PL], sflat[:, PL : 2 * PL])
        nc.sync.dma_start(out=out_d[g], in_=out_sb[:, 0:NO])
        yield


def _build():
    """Build + compile the SPMD Bass program (cached per process)."""
    if "nc" in _built:
        return _built["nc"]

    nc = bacc.Bacc(
        "TRN2", target_bir_lowering=False, debug=False, num_devices=NCORES
    )
    x_d = nc.dram_tensor("x", [G, 128, NX], mybir.dt.bfloat16, kind="ExternalInput").ap()
    zd_d = nc.dram_tensor(
        "zd", [G_PE, 128, NT, 128], mybir.dt.bfloat16, kind="ExternalInput"
    ).ap()
    zf_d = nc.dram_tensor(
        "zf", [128, G - G_PE, NT], mybir.dt.float32, kind="ExternalInput"
    ).ap()
    # bf16 outputs for PE/AG groups, fp32 for the DVE fp32 accumulators
    out_d = nc.dram_tensor("out", [G, 128, NO], mybir.dt.bfloat16, kind="ExternalOutput").ap()
    outf_d = nc.dram_tensor("outf", [G_DVE, 128, NO], mybir.dt.float32, kind="ExternalOutput").ap()

    with tile.TileContext(nc) as tc:
        with (
            tc.tile_pool(name="xs", bufs=4) as xs,
            tc.tile_pool(name="zs", bufs=3) as zs,
            tc.tile_pool(name="ws", bufs=2) as ws,
            tc.tile_pool(name="psA", bufs=4, space="PSUM") as psA,
            tc.tile_pool(name="psB", bufs=4, space="PSUM") as psB,
        ):
            pools = dict(xp=xs, zp=zs, op=ws, xv=xs, ov=ws,
                         xa=xs, ta=ws, sa=ws, oa=ws, zc=zs, psA=psA, psB=psB)
            x_flat = x_d.rearrange("g p n -> (g p n)")

            # HAM warmup: ~10 matmuls on scratch data keep the PE busy
            # during the first DMA wait so the real matmuls start at
            # 2.4 GHz instead of paying the 1.2 GHz cold window
            warm_x = zs.tile([128, 500], mybir.dt.bfloat16, name="warmx")
            nc.vector.memset(warm_x, 0.0)
            warm_ps = psA.tile([128, 500], mybir.dt.float32, name="warmps", tag="pA", bufs=4)
            for _ in range(10):
                nc.tensor.matmul(
                    warm_ps, warm_x[:, 0:128], warm_x, start=True, stop=True
                )

            zf_sb = zs.tile([128, G - G_PE, NT], mybir.dt.float32, name="zf")

            dve_gen = _gen_dve_groups(
                nc, pools, x_flat, zf_sb, outf_d, range(G_PE, G_PE + G_DVE)
            )
            ag_gen = _gen_ag_groups(
                nc, pools, x_d, zf_sb, out_d, range(G_PE + G_DVE, G)
            )
            from collections import deque

            # total yields: AG 6 groups x (6 chunks x 2 + 1), DVE 4 x (12 + 1)
            TOT_AG = G_AG * 13
            TOT_DVE = G_DVE * 13
            ag_n = dve_n = 0
            pending_evacs = deque()
            for g in range(G_PE):
                pA, pB = _emit_pe_matmuls(nc, pools, x_d, zd_d, g)
                if g == 0:
                    # z scalars for the elementwise groups, queued after
                    # group 0's loads so they don't delay the first matmul
                    nc.sync.dma_start(out=zf_sb, in_=zf_d)
                pending_evacs.append((g, pA, pB))
                # psum pools hold 4 tiles; evac lags by 1 group and is
                # emitted BEFORE this iteration's AG products so it never
                # queues behind them on ScalarE
                if len(pending_evacs) > 1:
                    _emit_pe_evac(nc, pools, out_d, *pending_evacs.popleft())
                # fractional pacing keeps both pipelines finishing with PE;
                # alternate ag/dve pumps so AG tree adds never head-block
                # the DVE queue while ACT products are still in flight
                t_ag = min(TOT_AG, -(-(g + 1) * TOT_AG * 115 // (100 * G_PE)))
                t_dve = min(TOT_DVE, -(-(g + 1) * TOT_DVE * 115 // (100 * G_PE)))
                while ag_n < t_ag or dve_n < t_dve:
                    if ag_n < t_ag:
                        next(ag_gen, None)
                        ag_n += 1
                    if dve_n < t_dve:
                        next(dve_gen, None)
                        dve_n += 1
            while pending_evacs:
                _emit_pe_evac(nc, pools, out_d, *pending_evacs.popleft())
            for _ in ag_gen:
                pass
            for _ in dve_gen:
                pass

    nc.compile()
    _built["nc"] = nc
    return nc


def _host_prep(z_f: np.ndarray, x_f: np.ndarray):
    """Shard + reformat inputs for the 8 cores."""
    x = np.ascontiguousarray(x_f, dtype=np.float32).reshape(B, C, NX)
    z = np.ascontiguousarray(z_f, dtype=np.float32).reshape(B, C, NT)
    in_maps = []
    p_idx = np.arange(128)
    for k in range(NCORES):
        xs = x[k * BPC : (k + 1) * BPC].reshape(G, 128, NX).astype(BF16)
        zs = z[k * BPC : (k + 1) * BPC].reshape(G, 128, NT)
        zd = np.zeros((G_PE, 128, NT, 128), dtype=BF16)
        # zd[g, p, t, p] = z[g*128+p, t]
        zd[:, p_idx, :, p_idx] = zs[:G_PE].astype(BF16).transpose(1, 0, 2)
        zfl = np.ascontiguousarray(zs[G_PE:].transpose(1, 0, 2))  # [128, 10, 49]
        in_maps.append({"x": xs, "zd": zd, "zf": zfl})
    return in_maps


def _run(z_f, x_f, trace=False, **spmd_kwargs):
    nc = _build()
    in_maps = _host_prep(z_f, x_f)
    if trace:
        _ensure_ntff_hook()
        # local profiling only — skip the artifact share upload
        import concourse.bass_utils as _bu

        _bu.upload_artifacts = lambda tmpdir: tmpdir
    res = None
    for attempt in range(3):
        try:
            res = run_bass_kernel_spmd(
                nc, in_maps, core_ids=list(range(NCORES)), trace=trace, **spmd_kwargs
            )
            break
        except Exception:
            # the device occasionally reports a transient unrecoverable
            # state on the first touch after another process exits;
            # re-running recovers it
            if attempt == 2:
                raise
            import time

            time.sleep(5)
    full = np.empty((B, C, HO, WO), np.float32)
    fv = full.reshape(NCORES, G, 128, NO)
    for k, r in enumerate(res.results):
        fv[k] = np.asarray(r["out"], dtype=np.float32)
        fv[k, G_PE : G_PE + G_DVE] = np.asarray(r["outf"], dtype=np.float32)
    return full, res


def _sane(full):
    # transiently wedged devices return garbage (|out| ~ 1e30); real outputs
    # for these inputs are O(sqrt(NT * log)) ~ tens
    return bool(np.isfinite(full).all() and np.abs(full).max() < 1e5)


def kernel(z_f: np.ndarray, x_f: np.ndarray) -> np.ndarray:
    for _ in range(3):
        full, _ = _run(z_f, x_f, trace=False)
        if _sane(full):
            return full
    return full
